# revision 8
# baseline (speedup 1.0000x reference)
"""Trainium2 Bass kernel for nn_CausalSelfAttention (B=1, T=2048, DIM=2048, H=16, D=128).

Strategy (8 NeuronCores, tensor-parallel over heads, 2 heads/core):
  - Host pre-transposes x -> xT [DIM, T] and slices/transposes the weights per core.
  - Per core: QKV projection time-major via fp32r matmuls (full-rate fp32),
    RMS-norm + RoPE in time-major layout (per-partition reductions),
    PE-transpose q,k to feature-major [d, t],
    attention computed as S^T = K Q^T blocks [tk,128 x tq,512] with causal
    block skipping; softmax WITHOUT max-subtraction (scores bounded by sqrt(128)
    since q,k are RMS-normed and RoPE preserves norms); row sums via ones-matmul
    on the PE; P^T V accumulated directly into y^T [d, tq] in PSUM.
  - c_proj partial products per core -> outT [DIM, T]; host sums the 8 partials
    (no on-device collectives) and transposes back.
"""

import sys

for _p in ("/opt/trn_rl_repo",):
    if _p not in sys.path:
        sys.path.append(_p)

from contextlib import ExitStack

import numpy as np

import concourse.bass as bass
import concourse.tile as tile
from concourse import bacc, mybir
from concourse.bass_utils import run_bass_kernel_spmd
from concourse.masks import make_identity

F32 = mybir.dt.float32
F32R = mybir.dt.float32r

B, T_FULL, DIM = 1, 2048, 2048
H, D = 16, 128
N_CORES = 8
HPC = H // N_CORES          # heads per core = 2
FQK = 2 * HPC * D           # 512  (q_h0|q_h1|k_h0|k_h1)
FV = HPC * D                # 256  (v_h0|v_h1)
EPS = float(np.finfo(np.float32).eps)
NEG = -60.0                 # additive causal mask (exp(-60+11.3) ~ 1e-22)


# --------------------------------------------------------------------------
# device kernel
# --------------------------------------------------------------------------
def _emit(ctx: ExitStack, tc: tile.TileContext, T: int, aps: dict):
    nc = tc.nc
    NTB = T // 128    # t-blocks
    NTT = T // 512    # t-tiles
    NKB = DIM // 128  # contraction blocks

    xT, wqk, wv, ve, cs, msk, wp, outT = (
        aps["xT"], aps["wqk"], aps["wv"], aps["ve"], aps["cs"], aps["msk"],
        aps["wp"], aps["outT"],
    )

    const = ctx.enter_context(tc.tile_pool(name="const", bufs=1))
    wpool = ctx.enter_context(tc.tile_pool(name="wpool", bufs=1))
    xpool = ctx.enter_context(tc.tile_pool(name="xpool", bufs=18))
    qkv = ctx.enter_context(tc.tile_pool(name="qkv", bufs=1))
    work = ctx.enter_context(tc.tile_pool(name="work", bufs=2))
    ps = ctx.enter_context(tc.tile_pool(name="ps", bufs=1, space="PSUM"))

    # ---- constants / weights resident in SBUF ----
    ident = const.tile([128, 128], F32)
    make_identity(nc, ident)
    ones_d = aps["ones"]
    ones_col = const.tile([128, 1], F32R)       # lhsT for row-sum matmul
    nc.sync.dma_start(out=ones_col, in_=ones_d[0, :].unsqueeze(1))
    ones_row = const.tile([1, 128], F32R)       # lhsT for broadcast matmul
    nc.sync.dma_start(out=ones_row, in_=ones_d[0:1, :])
    mask_sb = const.tile([128, 128], F32)
    nc.sync.dma_start(out=mask_sb, in_=msk)
    eps_q = const.tile([128, 1], F32)
    nc.vector.memset(eps_q, float(D * EPS))
    eps_k = const.tile([128, 1], F32)
    nc.vector.memset(eps_k, EPS)
    cs_sb = const.tile([128, NTB, 64], F32)
    nc.sync.dma_start(out=cs_sb, in_=cs.rearrange("(tb p) c -> p tb c", p=128))

    wqk_sb = wpool.tile([128, NKB, FQK], F32R)
    nc.sync.dma_start(out=wqk_sb, in_=wqk.rearrange("(kb p) f -> p kb f", p=128))
    wv_sb = wpool.tile([128, NKB, FV], F32R)
    nc.sync.dma_start(out=wv_sb, in_=wv.rearrange("(kb p) f -> p kb f", p=128))
    wp_sb = wpool.tile([128, HPC, DIM], F32R)
    nc.sync.dma_start(out=wp_sb, in_=wp.rearrange("(h p) c -> p h c", p=128))

    # ---- persistent activations ----
    # q^T,k^T feature-major per head: [d=128, t] ; v time-major [t, hpc*128]
    qT = [qkv.tile([128, NTB, 128], F32R, name=f"qT{h}") for h in range(HPC)]
    kT = [qkv.tile([128, NTB, 128], F32R, name=f"kT{h}") for h in range(HPC)]
    v_sb = qkv.tile([128, NTB, FV], F32R)

    # ==================================================================
    # Phase 1: QKV projection + rmsnorm + rope + transposes
    # ==================================================================
    for j in range(NTT):
        xkb = []
        for kb in range(NKB):
            xk = xpool.tile([128, 512], F32R, tag="xkb", name=f"x_{j}_{kb}")
            nc.sync.dma_start(
                out=xk, in_=xT[kb * 128:(kb + 1) * 128, j * 512:(j + 1) * 512]
            )
            xkb.append(xk)

        for tbl in range(4):
            tb = j * 4 + tbl
            tsl = slice(tbl * 128, (tbl + 1) * 128)

            qk_ps = ps.tile([128, FQK], F32, tag="mmA", bufs=3, name="qk_ps")
            for kb in range(NKB):
                nc.tensor.matmul(
                    qk_ps, xkb[kb][:, tsl], wqk_sb[:, kb, :],
                    start=(kb == 0), stop=(kb == NKB - 1),
                )
            v_ps = ps.tile([128, FV], F32, tag="mmB", bufs=2, name="v_ps")
            for kb in range(NKB):
                nc.tensor.matmul(
                    v_ps, xkb[kb][:, tsl], wv_sb[:, kb, :],
                    start=(kb == 0), stop=(kb == NKB - 1),
                )

            # v = (lam0*Wv) x + (lam1*ve)   (lambdas folded on host)
            vet = work.tile([128, FV], F32R, tag="vet")
            nc.sync.dma_start(out=vet, in_=ve[tb * 128:(tb + 1) * 128, :])
            nc.vector.tensor_add(v_sb[:, tb, :], v_ps, vet)

            # rms-norm per 128-wide head-half (q_h0 q_h1 k_h0 k_h1)
            qn = work.tile([128, FQK], F32, tag="qn")
            for hh in range(4):
                hsl = slice(hh * 128, (hh + 1) * 128)
                sq = work.tile([128, 128], F32, tag="sq")
                ss = work.tile([128, 1], F32, tag="ss")
                nc.scalar.activation(
                    sq, qk_ps[:, hsl], mybir.ActivationFunctionType.Square,
                    accum_out=ss,
                )
                rstd = work.tile([128, 1], F32, tag="rstd")
                if hh < 2:  # q: fold attention scale D**-0.5 into the rstd
                    nc.scalar.activation(
                        rstd, ss, mybir.ActivationFunctionType.Sqrt,
                        bias=eps_q, scale=1.0,
                    )
                else:       # k: plain rms
                    nc.scalar.activation(
                        rstd, ss, mybir.ActivationFunctionType.Sqrt,
                        bias=eps_k, scale=1.0 / D,
                    )
                nc.vector.reciprocal(rstd, rstd)
                nc.vector.tensor_scalar_mul(qn[:, hsl], qk_ps[:, hsl], rstd)

            # rope on dims [0:32] (paired with [64:96]) for all 4 head-halves
            qn4 = qn.rearrange("p (hh d) -> p hh d", hh=4)
            x1 = qn4[:, :, 0:32]
            x2 = qn4[:, :, 64:96]
            cos = cs_sb[:, tb, 0:32].unsqueeze(1).to_broadcast([128, 4, 32])
            sin = cs_sb[:, tb, 32:64].unsqueeze(1).to_broadcast([128, 4, 32])
            t1 = work.tile([128, 4, 32], F32, tag="t1")
            t2 = work.tile([128, 4, 32], F32, tag="t2")
            t3 = work.tile([128, 4, 32], F32, tag="t3")
            nc.vector.tensor_mul(t1, x1, cos)
            nc.vector.tensor_mul(t2, x2, sin)
            nc.vector.tensor_mul(t3, x1, sin)
            nc.vector.tensor_add(x1, t1, t2)         # x1' = x1*c + x2*s
            nc.vector.tensor_mul(t1, x2, cos)
            nc.vector.tensor_sub(x2, t1, t3)         # x2' = x2*c - x1*s

            # transpose q,k of both heads to feature-major
            for hh in range(4):
                hsl = slice(hh * 128, (hh + 1) * 128)
                tp = ps.tile([128, 128], F32, tag="mmB", bufs=2, name="tp_ps")
                nc.tensor.transpose(tp, qn[:, hsl], ident)
                dst = qT[hh][:, tb, :] if hh < 2 else kT[hh - 2][:, tb, :]
                nc.vector.tensor_copy(dst, tp)

    # ==================================================================
    # Phase 2+3: attention per (tq-tile, head), then c_proj partial
    # ==================================================================
    for j in range(NTT):
        yts = []
        for h in range(HPC):
            ilast = 4 * j + 3
            yT_ps = ps.tile([128, 512], F32, tag="acc", bufs=2, name="yT_ps")
            rs_ps = ps.tile([1, 512], F32, tag="rs", bufs=1, name="rs_ps")
            for i in range(ilast + 1):
                r = i - 4 * j
                c0 = max(r, 0) * 128
                csl = slice(c0, 512)
                s_ps = ps.tile([128, 512], F32, tag="mmA", bufs=3, name="s_ps")
                nc.tensor.matmul(
                    s_ps[:, csl],
                    kT[h][:, i, :],
                    qT[h][:, 4 * j + max(r, 0): 4 * j + 4, :],
                    start=True, stop=True,
                )
                if r >= 0:
                    dsl = slice(c0, c0 + 128)
                    nc.vector.tensor_add(s_ps[:, dsl], s_ps[:, dsl], mask_sb)
                p_sb = work.tile([128, 512], F32R, tag="p_sb", bufs=3)
                nc.scalar.activation(
                    p_sb[:, csl], s_ps[:, csl], mybir.ActivationFunctionType.Exp
                )
                nc.tensor.matmul(
                    yT_ps[:, csl], v_sb[:, i, h * 128:(h + 1) * 128], p_sb[:, csl],
                    start=(i == 0), stop=(i == ilast),
                )
                nc.tensor.matmul(
                    rs_ps[:, csl], ones_col, p_sb[:, csl],
                    start=(i == 0), stop=(i == ilast),
                )

            rs_sb = work.tile([1, 512], F32R, tag="rs_sb")
            with nc.allow_low_precision(reason="f32r rounding of softmax denom"):
                nc.vector.reciprocal(rs_sb, rs_ps)
            bc_ps = ps.tile([128, 512], F32, tag="mmB", bufs=2, name="bc_ps")
            nc.tensor.matmul(bc_ps, ones_row, rs_sb, start=True, stop=True)
            bc_sb = work.tile([128, 512], F32, tag="bc_sb")
            nc.vector.tensor_copy(bc_sb, bc_ps)
            yT_sb = work.tile([128, 512], F32R, tag="yT_sb", bufs=4)
            nc.vector.tensor_mul(yT_sb, yT_ps, bc_sb)
            yts.append(yT_sb)

        for cb in range(NKB):
            o_ps = ps.tile([128, 512], F32, tag="mmA", bufs=3, name="o_ps")
            for h in range(HPC):
                nc.tensor.matmul(
                    o_ps, wp_sb[:, h, cb * 128:(cb + 1) * 128], yts[h],
                    start=(h == 0), stop=(h == HPC - 1),
                )
            o_sb = work.tile([128, 512], F32, tag="o_sb", bufs=3)
            nc.vector.tensor_copy(o_sb, o_ps)
            nc.sync.dma_start(
                out=outT[cb * 128:(cb + 1) * 128, j * 512:(j + 1) * 512], in_=o_sb
            )


def _build(T: int):
    nc = bacc.Bacc("TRN2", target_bir_lowering=False, debug=False,
                   num_devices=N_CORES)
    aps = {
        "xT": nc.dram_tensor("xT", [DIM, T], F32R, kind="ExternalInput").ap(),
        "wqk": nc.dram_tensor("wqk", [DIM, FQK], F32R, kind="ExternalInput").ap(),
        "wv": nc.dram_tensor("wv", [DIM, FV], F32R, kind="ExternalInput").ap(),
        "ve": nc.dram_tensor("ve", [T, FV], F32R, kind="ExternalInput").ap(),
        "cs": nc.dram_tensor("cs", [T, 64], F32, kind="ExternalInput").ap(),
        "msk": nc.dram_tensor("msk", [128, 128], F32, kind="ExternalInput").ap(),
        "ones": nc.dram_tensor("ones", [2, 128], F32R, kind="ExternalInput").ap(),
        "wp": nc.dram_tensor("wp", [HPC * D, DIM], F32R, kind="ExternalInput").ap(),
        "outT": nc.dram_tensor("outT", [DIM, T], F32, kind="ExternalOutput").ap(),
    }
    with tile.TileContext(nc) as tc, ExitStack() as ctx:
        _emit(ctx, tc, T, aps)
    nc.compile()
    return nc


_NC_CACHE: dict = {}


def _get_nc(T: int):
    if T not in _NC_CACHE:
        _NC_CACHE[T] = _build(T)
    return _NC_CACHE[T]


# --------------------------------------------------------------------------
# host side
# --------------------------------------------------------------------------
def make_in_maps(x, ve, qkv_w, lambdas, c_proj_w):
    """Shard + pre-transpose full inputs into 8 per-core input maps."""
    T = x.shape[1]
    x2 = np.asarray(x, np.float32).reshape(T, DIM)
    xT = np.ascontiguousarray(x2.T)
    lam0, lam1 = float(lambdas[0]), float(lambdas[1])
    W = np.asarray(qkv_w, np.float32)
    vef = np.asarray(ve, np.float32).reshape(T, H, D)
    cw = np.asarray(c_proj_w, np.float32)

    # rope tables (only the 32 non-zero freqs rotate)
    freqs = (1.0 / 1024.0) ** np.linspace(0.0, 1.0, D // 4, dtype=np.float32)[:32]
    theta = np.outer(np.arange(T, dtype=np.float32), freqs)
    cs = np.concatenate([np.cos(theta), np.sin(theta)], axis=1).astype(np.float32)

    pm = np.arange(128)[:, None]
    qm = np.arange(128)[None, :]
    msk = np.where(pm <= qm, 0.0, NEG).astype(np.float32)

    in_maps = []
    for c in range(N_CORES):
        h0, h1 = HPC * c, HPC * c + 1
        wqk = np.concatenate(
            [W[0, h0 * D:(h0 + 1) * D], W[0, h1 * D:(h1 + 1) * D],
             W[1, h0 * D:(h0 + 1) * D], W[1, h1 * D:(h1 + 1) * D]], axis=0
        ).T
        wv = (lam0 * np.concatenate(
            [W[2, h0 * D:(h0 + 1) * D], W[2, h1 * D:(h1 + 1) * D]], axis=0)).T
        vec = lam1 * vef[:, h0:h1 + 1, :].reshape(T, FV)
        wp = cw[:, h0 * D:(h1 + 1) * D].T
        in_maps.append({
            "xT": xT,
            "wqk": np.ascontiguousarray(wqk, np.float32),
            "wv": np.ascontiguousarray(wv, np.float32),
            "ve": np.ascontiguousarray(vec, np.float32),
            "cs": cs,
            "msk": msk,
            "ones": np.ones((2, 128), np.float32),
            "wp": np.ascontiguousarray(wp, np.float32),
        })
    return in_maps


def combine_outputs(results, T):
    acc = results[0]["outT"].astype(np.float32)
    for r in results[1:]:
        acc = acc + r["outT"]
    return np.ascontiguousarray(acc.T).reshape(1, T, DIM)


def kernel(x, ve, block_mask, qkv_w, lambdas, c_proj_w):
    T = x.shape[1]
    nc = _get_nc(T)
    in_maps = make_in_maps(x, ve, qkv_w, lambdas, c_proj_w)
    res = run_bass_kernel_spmd(nc, in_maps, core_ids=list(range(N_CORES)))
    return combine_outputs(res.results, T)


# revision 11
# speedup vs baseline: 15.3343x; 15.3343x over previous
"""Trainium2 Bass kernel for nn_CausalSelfAttention (B=1, T=2048, DIM=2048, H=16, D=128).

Strategy (8 NeuronCores, tensor-parallel over heads, 2 heads/core):
  - Host pre-transposes x -> xT [DIM, T] and slices/transposes the weights per core.
  - Per core: QKV projection time-major via fp32r matmuls (full-rate fp32),
    RMS-norm + RoPE in time-major layout (per-partition reductions),
    PE-transpose q,k to feature-major [d, t],
    attention computed as S^T = K Q^T blocks [tk,128 x tq,512] with causal
    block skipping; softmax WITHOUT max-subtraction (scores bounded by sqrt(128)
    since q,k are RMS-normed and RoPE preserves norms); row sums via ones-matmul
    on the PE; P^T V accumulated directly into y^T [d, tq] in PSUM.
  - c_proj partial products per core -> outT [DIM, T]; host sums the 8 partials
    (no on-device collectives) and transposes back.
"""

import sys

for _p in ("/opt/trn_rl_repo",):
    if _p not in sys.path:
        sys.path.append(_p)

from contextlib import ExitStack

import numpy as np

import concourse.bass as bass
import concourse.tile as tile
from concourse import bacc, mybir
from concourse.bass_utils import run_bass_kernel_spmd
from concourse.masks import make_identity

F32 = mybir.dt.float32
F32R = mybir.dt.float32r

B, T_FULL, DIM = 1, 2048, 2048
H, D = 16, 128
N_CORES = 8
HPC = H // N_CORES          # heads per core = 2
FQK = 2 * HPC * D           # 512  (q_h0|q_h1|k_h0|k_h1)
FV = HPC * D                # 256  (v_h0|v_h1)
EPS = float(np.finfo(np.float32).eps)
NEG = -60.0                 # additive causal mask (exp(-60+11.3) ~ 1e-22)


# --------------------------------------------------------------------------
# device kernel
# --------------------------------------------------------------------------
def _emit(ctx: ExitStack, tc: tile.TileContext, T: int, aps: dict, iters: int = 1):
    if iters > 1:
        with tc.For_i(0, iters, 1):
            _emit_body(ctx, tc, T, aps)
    else:
        _emit_body(ctx, tc, T, aps)


def _emit_body(ctx: ExitStack, tc: tile.TileContext, T: int, aps: dict):
    nc = tc.nc
    NTB = T // 128    # t-blocks
    NTT = T // 512    # t-tiles
    NKB = DIM // 128  # contraction blocks

    xT, wqk, wv, ve, cs, msk, wp, outT = (
        aps["xT"], aps["wqk"], aps["wv"], aps["ve"], aps["cs"], aps["msk"],
        aps["wp"], aps["outT"],
    )

    const = ctx.enter_context(tc.tile_pool(name="const", bufs=1))
    wpool = ctx.enter_context(tc.tile_pool(name="wpool", bufs=1))
    xpool = ctx.enter_context(tc.tile_pool(name="xpool", bufs=18))
    qkv = ctx.enter_context(tc.tile_pool(name="qkv", bufs=1))
    work = ctx.enter_context(tc.tile_pool(name="work", bufs=2))
    ps = ctx.enter_context(tc.tile_pool(name="ps", bufs=1, space="PSUM"))

    # ---- constants / weights resident in SBUF ----
    ident = const.tile([128, 128], F32)
    make_identity(nc, ident)
    ones_d = aps["ones"]
    ones_col = const.tile([128, 1], F32R)       # lhsT for row-sum matmul
    nc.sync.dma_start(out=ones_col, in_=ones_d[0, :].unsqueeze(1))
    ones_row = const.tile([1, 128], F32R)       # lhsT for broadcast matmul
    nc.sync.dma_start(out=ones_row, in_=ones_d[0:1, :])
    mask_sb = const.tile([128, 128], F32)
    nc.sync.dma_start(out=mask_sb, in_=msk)
    eps_q = const.tile([128, 1], F32)
    nc.vector.memset(eps_q, float(D * EPS))
    eps_k = const.tile([128, 1], F32)
    nc.vector.memset(eps_k, EPS)
    cs_sb = const.tile([128, NTB, 64], F32)
    nc.sync.dma_start(out=cs_sb, in_=cs.rearrange("(tb p) c -> p tb c", p=128))

    wqk_sb = wpool.tile([128, NKB, FQK], F32R)
    nc.sync.dma_start(out=wqk_sb, in_=wqk.rearrange("(kb p) f -> p kb f", p=128))
    wv_sb = wpool.tile([128, NKB, FV], F32R)
    nc.sync.dma_start(out=wv_sb, in_=wv.rearrange("(kb p) f -> p kb f", p=128))
    wp_sb = wpool.tile([128, HPC, DIM], F32R)
    nc.sync.dma_start(out=wp_sb, in_=wp.rearrange("(h p) c -> p h c", p=128))

    # ---- persistent activations ----
    # q^T,k^T feature-major per head: [d=128, t] ; v time-major [t, hpc*128]
    qT = [qkv.tile([128, NTB, 128], F32R, name=f"qT{h}") for h in range(HPC)]
    kT = [qkv.tile([128, NTB, 128], F32R, name=f"kT{h}") for h in range(HPC)]
    v_sb = qkv.tile([128, NTB, FV], F32R)

    # ==================================================================
    # Phase 1: QKV projection + rmsnorm + rope + transposes
    # ==================================================================
    for j in range(NTT):
        xkb = []
        for kb in range(NKB):
            xk = xpool.tile([128, 512], F32R, tag="xkb", name=f"x_{j}_{kb}")
            nc.sync.dma_start(
                out=xk, in_=xT[kb * 128:(kb + 1) * 128, j * 512:(j + 1) * 512]
            )
            xkb.append(xk)

        for tbl in range(4):
            tb = j * 4 + tbl
            tsl = slice(tbl * 128, (tbl + 1) * 128)

            qk_ps = ps.tile([128, FQK], F32, tag="mmA", bufs=3, name="qk_ps")
            for kb in range(NKB):
                nc.tensor.matmul(
                    qk_ps, xkb[kb][:, tsl], wqk_sb[:, kb, :],
                    start=(kb == 0), stop=(kb == NKB - 1),
                )
            v_ps = ps.tile([128, FV], F32, tag="mmB", bufs=2, name="v_ps")
            for kb in range(NKB):
                nc.tensor.matmul(
                    v_ps, xkb[kb][:, tsl], wv_sb[:, kb, :],
                    start=(kb == 0), stop=(kb == NKB - 1),
                )

            # v = (lam0*Wv) x + (lam1*ve)   (lambdas folded on host)
            vet = work.tile([128, FV], F32R, tag="vet")
            nc.sync.dma_start(out=vet, in_=ve[tb * 128:(tb + 1) * 128, :])
            nc.vector.tensor_add(v_sb[:, tb, :], v_ps, vet)

            # rms-norm per 128-wide head-half (q_h0 q_h1 k_h0 k_h1)
            qn = work.tile([128, FQK], F32, tag="qn")
            for hh in range(4):
                hsl = slice(hh * 128, (hh + 1) * 128)
                sq = work.tile([128, 128], F32, tag="sq")
                ss = work.tile([128, 1], F32, tag="ss")
                nc.scalar.activation(
                    sq, qk_ps[:, hsl], mybir.ActivationFunctionType.Square,
                    accum_out=ss,
                )
                rstd = work.tile([128, 1], F32, tag="rstd")
                if hh < 2:  # q: fold attention scale D**-0.5 into the rstd
                    nc.scalar.activation(
                        rstd, ss, mybir.ActivationFunctionType.Sqrt,
                        bias=eps_q, scale=1.0,
                    )
                else:       # k: plain rms
                    nc.scalar.activation(
                        rstd, ss, mybir.ActivationFunctionType.Sqrt,
                        bias=eps_k, scale=1.0 / D,
                    )
                nc.vector.reciprocal(rstd, rstd)
                nc.vector.tensor_scalar_mul(qn[:, hsl], qk_ps[:, hsl], rstd)

            # rope on dims [0:32] (paired with [64:96]) for all 4 head-halves
            qn4 = qn.rearrange("p (hh d) -> p hh d", hh=4)
            x1 = qn4[:, :, 0:32]
            x2 = qn4[:, :, 64:96]
            cos = cs_sb[:, tb, 0:32].unsqueeze(1).to_broadcast([128, 4, 32])
            sin = cs_sb[:, tb, 32:64].unsqueeze(1).to_broadcast([128, 4, 32])
            t1 = work.tile([128, 4, 32], F32, tag="t1")
            t2 = work.tile([128, 4, 32], F32, tag="t2")
            t3 = work.tile([128, 4, 32], F32, tag="t3")
            nc.vector.tensor_mul(t1, x1, cos)
            nc.vector.tensor_mul(t2, x2, sin)
            nc.vector.tensor_mul(t3, x1, sin)
            nc.vector.tensor_add(x1, t1, t2)         # x1' = x1*c + x2*s
            nc.vector.tensor_mul(t1, x2, cos)
            nc.vector.tensor_sub(x2, t1, t3)         # x2' = x2*c - x1*s

            # transpose q,k of both heads to feature-major
            for hh in range(4):
                hsl = slice(hh * 128, (hh + 1) * 128)
                tp = ps.tile([128, 128], F32, tag="mmB", bufs=2, name="tp_ps")
                nc.tensor.transpose(tp, qn[:, hsl], ident)
                dst = qT[hh][:, tb, :] if hh < 2 else kT[hh - 2][:, tb, :]
                nc.vector.tensor_copy(dst, tp)

    # ==================================================================
    # Phase 2+3: attention per (tq-tile, head), then c_proj partial
    # ==================================================================
    for j in range(NTT):
        yts = []
        for h in range(HPC):
            ilast = 4 * j + 3
            yT_ps = ps.tile([128, 512], F32, tag="acc", bufs=2, name="yT_ps")
            rs_ps = ps.tile([1, 512], F32, tag="rs", bufs=1, name="rs_ps")
            for i in range(ilast + 1):
                r = i - 4 * j
                c0 = max(r, 0) * 128
                csl = slice(c0, 512)
                s_ps = ps.tile([128, 512], F32, tag="mmA", bufs=3, name="s_ps")
                nc.tensor.matmul(
                    s_ps[:, csl],
                    kT[h][:, i, :],
                    qT[h][:, 4 * j + max(r, 0): 4 * j + 4, :],
                    start=True, stop=True,
                )
                if r >= 0:
                    dsl = slice(c0, c0 + 128)
                    nc.vector.tensor_add(s_ps[:, dsl], s_ps[:, dsl], mask_sb)
                p_sb = work.tile([128, 512], F32R, tag="p_sb", bufs=3)
                nc.scalar.activation(
                    p_sb[:, csl], s_ps[:, csl], mybir.ActivationFunctionType.Exp
                )
                nc.tensor.matmul(
                    yT_ps[:, csl], v_sb[:, i, h * 128:(h + 1) * 128], p_sb[:, csl],
                    start=(i == 0), stop=(i == ilast),
                )
                nc.tensor.matmul(
                    rs_ps[:, csl], ones_col, p_sb[:, csl],
                    start=(i == 0), stop=(i == ilast),
                )

            rs_sb = work.tile([1, 512], F32R, tag="rs_sb")
            with nc.allow_low_precision(reason="f32r rounding of softmax denom"):
                nc.vector.reciprocal(rs_sb, rs_ps)
            bc_ps = ps.tile([128, 512], F32, tag="mmB", bufs=2, name="bc_ps")
            nc.tensor.matmul(bc_ps, ones_row, rs_sb, start=True, stop=True)
            bc_sb = work.tile([128, 512], F32, tag="bc_sb")
            nc.vector.tensor_copy(bc_sb, bc_ps)
            yT_sb = work.tile([128, 512], F32R, tag="yT_sb", bufs=4)
            nc.vector.tensor_mul(yT_sb, yT_ps, bc_sb)
            yts.append(yT_sb)

        for cb in range(NKB):
            o_ps = ps.tile([128, 512], F32, tag="mmA", bufs=3, name="o_ps")
            for h in range(HPC):
                nc.tensor.matmul(
                    o_ps, wp_sb[:, h, cb * 128:(cb + 1) * 128], yts[h],
                    start=(h == 0), stop=(h == HPC - 1),
                )
            o_sb = work.tile([128, 512], F32, tag="o_sb", bufs=3)
            nc.vector.tensor_copy(o_sb, o_ps)
            nc.sync.dma_start(
                out=outT[cb * 128:(cb + 1) * 128, j * 512:(j + 1) * 512], in_=o_sb
            )


def _build(T: int, iters: int = 1):
    nc = bacc.Bacc("TRN2", target_bir_lowering=False, debug=False,
                   num_devices=N_CORES)
    aps = {
        "xT": nc.dram_tensor("xT", [DIM, T], F32R, kind="ExternalInput").ap(),
        "wqk": nc.dram_tensor("wqk", [DIM, FQK], F32R, kind="ExternalInput").ap(),
        "wv": nc.dram_tensor("wv", [DIM, FV], F32R, kind="ExternalInput").ap(),
        "ve": nc.dram_tensor("ve", [T, FV], F32R, kind="ExternalInput").ap(),
        "cs": nc.dram_tensor("cs", [T, 64], F32, kind="ExternalInput").ap(),
        "msk": nc.dram_tensor("msk", [128, 128], F32, kind="ExternalInput").ap(),
        "ones": nc.dram_tensor("ones", [2, 128], F32R, kind="ExternalInput").ap(),
        "wp": nc.dram_tensor("wp", [HPC * D, DIM], F32R, kind="ExternalInput").ap(),
        "outT": nc.dram_tensor("outT", [DIM, T], F32, kind="ExternalOutput").ap(),
    }
    with tile.TileContext(nc) as tc, ExitStack() as ctx:
        _emit(ctx, tc, T, aps, iters=iters)
    nc.compile()
    return nc


_NC_CACHE: dict = {}


def _get_nc(T: int, iters: int = 1):
    key = (T, iters)
    if key not in _NC_CACHE:
        _NC_CACHE[key] = _build(T, iters)
    return _NC_CACHE[key]


# --------------------------------------------------------------------------
# host side
# --------------------------------------------------------------------------
def make_in_maps(x, ve, qkv_w, lambdas, c_proj_w):
    """Shard + pre-transpose full inputs into 8 per-core input maps."""
    T = x.shape[1]
    x2 = np.asarray(x, np.float32).reshape(T, DIM)
    xT = np.ascontiguousarray(x2.T)
    lam0, lam1 = float(lambdas[0]), float(lambdas[1])
    W = np.asarray(qkv_w, np.float32)
    vef = np.asarray(ve, np.float32).reshape(T, H, D)
    cw = np.asarray(c_proj_w, np.float32)

    # rope tables (only the 32 non-zero freqs rotate)
    freqs = (1.0 / 1024.0) ** np.linspace(0.0, 1.0, D // 4, dtype=np.float32)[:32]
    theta = np.outer(np.arange(T, dtype=np.float32), freqs)
    cs = np.concatenate([np.cos(theta), np.sin(theta)], axis=1).astype(np.float32)

    pm = np.arange(128)[:, None]
    qm = np.arange(128)[None, :]
    msk = np.where(pm <= qm, 0.0, NEG).astype(np.float32)

    in_maps = []
    for c in range(N_CORES):
        h0, h1 = HPC * c, HPC * c + 1
        wqk = np.concatenate(
            [W[0, h0 * D:(h0 + 1) * D], W[0, h1 * D:(h1 + 1) * D],
             W[1, h0 * D:(h0 + 1) * D], W[1, h1 * D:(h1 + 1) * D]], axis=0
        ).T
        wv = (lam0 * np.concatenate(
            [W[2, h0 * D:(h0 + 1) * D], W[2, h1 * D:(h1 + 1) * D]], axis=0)).T
        vec = lam1 * vef[:, h0:h1 + 1, :].reshape(T, FV)
        wp = cw[:, h0 * D:(h1 + 1) * D].T
        in_maps.append({
            "xT": xT,
            "wqk": np.ascontiguousarray(wqk, np.float32),
            "wv": np.ascontiguousarray(wv, np.float32),
            "ve": np.ascontiguousarray(vec, np.float32),
            "cs": cs,
            "msk": msk,
            "ones": np.ones((2, 128), np.float32),
            "wp": np.ascontiguousarray(wp, np.float32),
        })
    return in_maps


def combine_outputs(results, T):
    acc = results[0]["outT"].astype(np.float32)
    for r in results[1:]:
        acc = acc + r["outT"]
    return np.ascontiguousarray(acc.T).reshape(1, T, DIM)


def kernel(x, ve, block_mask, qkv_w, lambdas, c_proj_w):
    T = x.shape[1]
    nc = _get_nc(T)
    in_maps = make_in_maps(x, ve, qkv_w, lambdas, c_proj_w)
    res = run_bass_kernel_spmd(nc, in_maps, core_ids=list(range(N_CORES)))
    return combine_outputs(res.results, T)
